# revision 5
# baseline (speedup 1.0000x reference)
"""Trainium2 Bass kernel for nn_Enhancement_77309412162.

Math reduction (from the reference):
  theta[b,n] = sum_c x[b,c,n]*theta_w[c] + theta_b        (per-sample matvec)
  g[b,n]     = sum_c x[b,c,n]*g_w[c] + g_b
  phi1[b,n]  = sum_c x1[b,c,n]*phi_w[c] + phi_b
  phi2[b,n]  = sum_c x2[b,c,n]*phi_w[c] + phi_b
  The (N,N) affinity matrices are rank-1, so
  y[b,n] = s_b * theta[b,n],  s_b = (b/N)*(a_c*<phi1,g> + (1-a_c)*<phi2,g>)
  wy[b,c,n] = W_w[c]*t[b,n] + W_b[c],  t = s_b*theta_b
  BN over (B,H,W):  mean[c] = W_w[c]*mu + W_b[c],  var[c] = W_w[c]^2*var_t
  where mu/var_t are the global scalar mean/var of t over all (b,n).
  out[b,c,n] = x[b,c,n] + alpha[c]*(t[b,n]-mu) + bn_b[c]
  with alpha[c] = bn_w[c]*W_w[c]/sqrt(W_w[c]^2*var_t + 1e-5).

Sharding: batch-parallel, one sample per core (B=8, 8 cores). The only
cross-core data is an allreduce of [sum(t), sum(t^2)] (8 bytes/core).

v2 design (driven by the v1 NTFF trace):
- The ncfw collective stream has a fixed init BARRIER (21->62us on the
  traced core) plus ~11us first-op gap: NO collective can start before
  ~73us no matter when it is triggered.  v1 triggered at 77us (late),
  behind a warmup AllGather, so the real gather ran 83-95us.  v2 drops
  the warmup and makes sure every core triggers well before the 73us
  slot: the real AG then runs ~73-81us, the earliest physically
  possible under this runtime.
- All bulk input loads moved to the two HWDGE rings (SP + ACT), x
  first, then x1/x2 interleaved in 512-column spatial chunks.  DVE
  casts fp32->bf16 chunk-by-chunk and the PE projection matmuls + DVE
  d-dots chase the chunks, so upair is ready ~3us after the last chunk
  lands (loads are HBM-bound at ~40us).  Q7/SWDGE carries only the
  tiny constant casts and the 8-byte cc_in bounce, so the trigger is
  never queued behind bulk descriptor generation (v1 lost ~12us there).
- theta broadcast to 128 partitions now uses a [2,128] selector matmul
  on PE into PSUM + ACT copy (v1 bounced 2.4MB through DRAM inside the
  load window).
- Tail (everything after the AG completes): stride-0 readback of the
  gathered pairs, a ~11-op DVE chain (n1 = bn_w*W_w*s is prehoisted),
  then the per-channel apply in 16 quarter tiles, ACT (scale*theta+
  bias) -> DVE (+x) -> stores alternating across BOTH HWDGE rings.
  v1's tail was 27us (one store ring, 3.4us readback, 2.8us lead-in);
  v2 targets ~18us.
"""

import os
import numpy as np

B, C, H, W = 8, 512, 48, 48
N = H * W            # 2304
P = 128
J = C // P           # 4 channel chunks
NCHUNKS = [(0, 512), (512, 512), (1024, 512), (1536, 512), (2048, 256)]
QUART = N // 4       # 576
NCORES = 8
BN_COUNT = float(B * N)

_cache = {}


def _build_nc():
    import concourse.bass as bass
    import concourse.bacc as bacc
    import concourse.tile as tile
    from concourse import mybir
    from contextlib import ExitStack

    f32 = mybir.dt.float32
    bf16 = mybir.dt.bfloat16
    Alu = mybir.AluOpType
    Act = mybir.ActivationFunctionType
    AxX = mybir.AxisListType.X

    nc = bacc.Bacc("TRN2", target_bir_lowering=False, debug=False,
                   enable_asserts=False, num_devices=NCORES)

    x_d = nc.dram_tensor("x", [C, N], f32, kind="ExternalInput").ap()
    x1_d = nc.dram_tensor("x1", [C, N], f32, kind="ExternalInput").ap()
    x2_d = nc.dram_tensor("x2", [C, N], f32, kind="ExternalInput").ap()
    thw_d = nc.dram_tensor("theta_w", [C], f32, kind="ExternalInput").ap()
    gw_d = nc.dram_tensor("g_w", [C], f32, kind="ExternalInput").ap()
    phw_d = nc.dram_tensor("phi_w", [C], f32, kind="ExternalInput").ap()
    thb_d = nc.dram_tensor("theta_b", [1], f32, kind="ExternalInput").ap()
    gb_d = nc.dram_tensor("g_b", [1], f32, kind="ExternalInput").ap()
    phb_d = nc.dram_tensor("phi_b", [1], f32, kind="ExternalInput").ap()
    ww_d = nc.dram_tensor("W_w", [C], f32, kind="ExternalInput").ap()
    bnw_d = nc.dram_tensor("bn_w", [C], f32, kind="ExternalInput").ap()
    bnb_d = nc.dram_tensor("bn_b", [C], f32, kind="ExternalInput").ap()
    a_d = nc.dram_tensor("a", [1], f32, kind="ExternalInput").ap()
    b_d = nc.dram_tensor("b", [1], f32, kind="ExternalInput").ap()
    out_d = nc.dram_tensor("out", [C, N], f32, kind="ExternalOutput").ap()

    with tile.TileContext(nc) as tc, ExitStack() as ctx:
        singles = ctx.enter_context(tc.tile_pool(name="singles", bufs=1))
        tmps = ctx.enter_context(tc.tile_pool(name="tmps", bufs=3))
        scr = ctx.enter_context(tc.tile_pool(name="scr", bufs=2))
        psproj = ctx.enter_context(tc.tile_pool(name="psproj", bufs=3, space="PSUM"))
        psbc = ctx.enter_context(tc.tile_pool(name="psbc", bufs=2, space="PSUM"))
        psr = ctx.enter_context(tc.tile_pool(name="psr", bufs=1, space="PSUM"))
        dram = ctx.enter_context(tc.tile_pool(name="dram", bufs=1, space="DRAM"))

        if int(os.environ.get("KERNEL_CC_WARM", "0")):
            # optional warm-up collective for A/B experiments (v2 default
            # off: with an early real trigger the warmup only pushes the
            # real gather out of the first post-barrier ncfw slot)
            warm_in = dram.tile([1, 2], f32, name="warm_in")
            warm_out = dram.tile([1, 2 * NCORES], f32, name="warm_out")
            nc.gpsimd.collective_compute(
                "AllGather", Alu.bypass,
                replica_groups=[list(range(NCORES))],
                ins=[warm_in.opt()], outs=[warm_out.opt()],
            )

        # ---- small constant loads; Q7/SWDGE only carries the bf16 weight
        # casts so the collective trigger is never stuck behind bulk
        # descriptor generation ----
        wxt = singles.tile([P, J, 2], bf16, name="wxt")  # [theta_w | g_w]
        nc.gpsimd.dma_start(out=wxt[:, :, 0],
                            in_=thw_d.rearrange("(j p) -> p j", p=P))
        nc.gpsimd.dma_start(out=wxt[:, :, 1],
                            in_=gw_d.rearrange("(j p) -> p j", p=P))
        wpt = singles.tile([P, J, 2], bf16, name="wpt")  # [phi_w | phi_w]
        nc.gpsimd.dma_start(out=wpt[:, :, 0],
                            in_=phw_d.rearrange("(j p) -> p j", p=P))
        nc.gpsimd.dma_start(out=wpt[:, :, 1],
                            in_=phw_d.rearrange("(j p) -> p j", p=P))

        def load_pj(ap_d, nm):
            t = singles.tile([P, J], f32, name=nm)
            nc.scalar.dma_start(out=t, in_=ap_d.rearrange("(j p) -> p j", p=P))
            return t

        ww = load_pj(ww_d, "ww")
        bnw = load_pj(bnw_d, "bnw")
        bnb = load_pj(bnb_d, "bnb")

        thgb = singles.tile([2, 1], f32, name="thgb")   # row0 theta_b, row1 g_b
        nc.scalar.dma_start(out=thgb[0:1, :], in_=thb_d[None, :])
        nc.scalar.dma_start(out=thgb[1:2, :], in_=gb_d[None, :])
        phb2 = singles.tile([2, 1], f32, name="phb2")
        nc.scalar.dma_start(out=phb2, in_=bass.AP(tensor=phb_d.tensor,
                                                  offset=phb_d.offset,
                                                  ap=[[0, 2], [1, 1]]))
        av128 = singles.tile([P, 1], f32, name="av128")
        nc.scalar.dma_start(out=av128, in_=bass.AP(tensor=a_d.tensor,
                                                   offset=a_d.offset,
                                                   ap=[[0, P], [1, 1]]))
        bv128 = singles.tile([P, 1], f32, name="bv128")
        nc.scalar.dma_start(out=bv128, in_=bass.AP(tensor=b_d.tensor,
                                                   offset=b_d.offset,
                                                   ap=[[0, P], [1, 1]]))

        # ---- bulk input streams: x first (it gates theta/g and hence the
        # d-dots), then x1/x2 interleaved per 512-column chunk; sync ring
        # carries j=0,1 and the ACT ring j=2,3 so each chunk completes on
        # both rings at about the same time ----
        x_tiles = [singles.tile([P, N], f32, name=f"xt{j}") for j in range(J)]
        for (n0, nsz) in NCHUNKS:
            for j in range(J):
                eng = nc.sync if j < 2 else nc.scalar
                eng.dma_start(out=x_tiles[j][:, n0:n0 + nsz],
                              in_=x_d[j * P:(j + 1) * P, n0:n0 + nsz])

        # x1/x2 rotate through a small fp32 landing pool (SBUF cannot hold
        # them fp32-resident); DVE casts them into persistent bf16 tiles
        # chunk-by-chunk right behind the ring
        xland = ctx.enter_context(tc.tile_pool(name="xland", bufs=10))
        xb1 = [singles.tile([P, N], bf16, name=f"xb1_{j}") for j in range(J)]
        xb2 = [singles.tile([P, N], bf16, name=f"xb2_{j}") for j in range(J)]
        for (n0, nsz) in NCHUNKS:
            for src_d, xb in ((x1_d, xb1), (x2_d, xb2)):
                for j in range(J):
                    eng = nc.sync if j < 2 else nc.scalar
                    land = xland.tile([P, 512], f32, name="land")
                    eng.dma_start(out=land[:, :nsz],
                                  in_=src_d[j * P:(j + 1) * P, n0:n0 + nsz])
                    nc.vector.tensor_copy(xb[j][:, n0:n0 + nsz],
                                          land[:, :nsz])

        # bf16 copy of x chases the chunk loads on DVE; fp32 x stays
        # resident for the exact residual add
        xbt = [singles.tile([P, N], bf16, name=f"xbt{j}") for j in range(J)]
        for (n0, nsz) in NCHUNKS:
            for j in range(J):
                nc.vector.tensor_copy(xbt[j][:, n0:n0 + nsz],
                                      x_tiles[j][:, n0:n0 + nsz])

        # selector lhsTs: sel0 broadcasts partition-0 values to all 128
        # partitions, sel1 broadcasts partition-1 values
        sel0 = singles.tile([2, P], f32, name="sel0")
        nc.vector.memset(sel0, 0.0)
        nc.vector.memset(sel0[0:1, :], 1.0)
        sel1 = singles.tile([2, P], f32, name="sel1")
        nc.vector.memset(sel1, 1.0)
        nc.vector.tensor_sub(sel1, sel1, sel0)
        sel0b = singles.tile([2, P], bf16, name="sel0b")
        nc.vector.memset(sel0b, 0.0)
        nc.vector.memset(sel0b[0:1, :], 1.0)

        # partials, written as (2,.) pairs; partner row is ignored:
        # col0 row0=A=sum(theta) row1=C=sum(g), col1 row0=B=sum(theta^2),
        # col2 row1=d1=<phi1,g>, col3 row1=d2=<phi2,g> (biases folded in)
        PT = singles.tile([2, 4], f32, name="PT")
        upair = singles.tile([P, 2], f32, name="upair")
        nc.vector.memset(upair, 0.0)

        # warm the sqrt ACT table set early so the post-exchange sqrt does
        # not pay the ~2.7us table load on the critical tail
        sqwarm = singles.tile([P, 1], f32, name="sqwarm")
        nc.scalar.activation(out=sqwarm, in_=av128, func=Act.Sqrt)

        # prehoisted per-channel products (collective-independent)
        ww2 = singles.tile([P, J], f32, name="ww2")
        nc.vector.tensor_mul(ww2, ww, ww)
        alw = singles.tile([P, J], f32, name="alw")
        nc.vector.tensor_mul(alw, bnw, ww)
        # s = (b/N)*(a_c*d1 + (1-a_c)*d2) = c1*d1 + c2*d2 with c1/c2
        # prehoisted from a,b alone so the post-dot chain is 3 ops
        ac = singles.tile([P, 1], f32, name="ac")
        nc.vector.tensor_scalar(ac, av128, 0.0, 1.0, op0=Alu.max, op1=Alu.min)
        c1 = singles.tile([P, 1], f32, name="c1")
        nc.vector.tensor_mul(c1, ac, bv128)
        nc.vector.tensor_scalar_mul(c1, c1, 1.0 / float(N))
        c2 = singles.tile([P, 1], f32, name="c2")
        nc.vector.tensor_scalar(c2, ac, 1.0, None, op0=Alu.subtract)
        nc.vector.tensor_mul(c2, c2, bv128)
        nc.vector.tensor_scalar_mul(c2, c2, -1.0 / float(N))

        # ---- theta/g projection (bf16 single-pass PE), chunk-chased ----
        thg = singles.tile([2, N], f32, name="thg")     # row0 theta, row1 g
        for (n0, nsz) in NCHUNKS:
            ps = psproj.tile([2, 512], f32, name="ps")
            for j in range(J):
                nc.tensor.matmul(ps[:, :nsz], lhsT=wxt[:, j, :],
                                 rhs=xbt[j][:, n0:n0 + nsz],
                                 start=(j == 0), stop=(j == J - 1))
            nc.scalar.activation(out=thg[:, n0:n0 + nsz], in_=ps[:, :nsz],
                                 func=Act.Identity, bias=thgb, scale=1.0)

        # theta broadcast to all 128 partitions: bf16 selector matmul into
        # PSUM + ACT copy to SBUF (no DRAM bounce, PE only)
        thgb16 = singles.tile([2, N], bf16, name="thgb16")
        theta_bc = singles.tile([P, N], f32, name="theta_bc")
        for (n0, nsz) in NCHUNKS:
            nc.vector.tensor_copy(thgb16[:, n0:n0 + nsz],
                                  thg[:, n0:n0 + nsz])
            pb = psbc.tile([P, 512], f32, name="pb")
            nc.tensor.matmul(pb[:, :nsz], lhsT=sel0b,
                             rhs=thgb16[:, n0:n0 + nsz],
                             start=True, stop=True)
            nc.scalar.activation(out=theta_bc[:, n0:n0 + nsz],
                                 in_=pb[:, :nsz], func=Act.Identity)

        # A = sum(theta) & C = sum(g) (accum rows 0/1), B = sum(theta^2)
        sq_scr = scr.tile([2, N], f32, name="sq_scr")
        nc.scalar.activation(out=sq_scr, in_=thg, func=Act.Identity,
                             accum_out=PT[:, 0:1])
        nc.scalar.activation(out=sq_scr, in_=thg, func=Act.Square,
                             accum_out=PT[:, 1:2])

        # ---- phi projections (bf16 single-pass PE) + chunked d-dots,
        # chasing the x1/x2 chunk loads ----
        phi1 = singles.tile([2, N], f32, name="phi1")
        phi2 = singles.tile([2, N], f32, name="phi2")
        dk1 = singles.tile([2, len(NCHUNKS)], f32, name="dk1")
        dk2 = singles.tile([2, len(NCHUNKS)], f32, name="dk2")
        for k, (n0, nsz) in enumerate(NCHUNKS):
            for (xb, phi, dk) in ((xb1, phi1, dk1), (xb2, phi2, dk2)):
                ps = psproj.tile([2, 512], f32, name="ps")
                for j in range(J):
                    nc.tensor.matmul(ps[:, :nsz], lhsT=wpt[:, j, :],
                                     rhs=xb[j][:, n0:n0 + nsz],
                                     start=(j == 0), stop=(j == J - 1))
                nc.scalar.activation(out=phi[:, n0:n0 + nsz], in_=ps[:, :nsz],
                                     func=Act.Identity, bias=phb2, scale=1.0)
                ds = scr.tile([2, 512], f32, name="d_scr")
                nc.vector.tensor_mul(ds[:, :nsz], phi[:, n0:n0 + nsz],
                                     thg[:, n0:n0 + nsz])
                nc.vector.tensor_reduce(dk[:, k:k + 1], ds[:, :nsz],
                                        axis=AxX, op=Alu.add)
        nc.vector.tensor_reduce(PT[:, 2:3], dk1, axis=AxX, op=Alu.add)
        nc.vector.tensor_reduce(PT[:, 3:4], dk2, axis=AxX, op=Alu.add)

        # broadcast partial rows to all 128 partitions; the DVE chain
        # reads the PSUM results directly
        pr = psr.tile([P, 4], f32, name="pr")
        nc.tensor.matmul(pr[:, 0:2], lhsT=sel0, rhs=PT[:, 0:2],
                         start=True, stop=True)
        nc.tensor.matmul(pr[:, 2:4], lhsT=sel1, rhs=PT[:, 2:4],
                         start=True, stop=True)
        A_ = pr[:, 0:1]
        B_ = pr[:, 1:2]
        d1_ = pr[:, 2:3]
        d2_ = pr[:, 3:4]

        # s = c1*d1 + c2*d2; u1 = s*A; u2 = s^2*B  (replicated)
        sv = singles.tile([P, 1], f32, name="sv")
        sv2p = singles.tile([P, 1], f32, name="sv2p")
        nc.vector.tensor_mul(sv, c1, d1_)
        nc.vector.tensor_mul(sv2p, c2, d2_)
        nc.vector.tensor_add(sv, sv, sv2p)
        s2v = singles.tile([P, 1], f32, name="s2v")
        nc.vector.tensor_mul(s2v, sv, sv)
        nc.vector.tensor_mul(upair[:, 0:1], sv, A_)
        nc.vector.tensor_mul(upair[:, 1:2], s2v, B_)
        # n1 = bn_w*W_w*s, prehoisted off the post-collective chain
        n1 = singles.tile([P, J], f32, name="n1")
        nc.vector.tensor_scalar(n1, alw, sv, None, op0=Alu.mult)

        # ---- 8-byte-per-core allreduce across the 8 cores (ncfw AG) ----
        cc_in = dram.tile([1, 2], f32, name="cc_in")
        cc_out = dram.tile([1, 2 * NCORES], f32, name="cc_out")
        nc.gpsimd.dma_start(out=cc_in, in_=upair[0:1, :])
        nc.gpsimd.collective_compute(
            "AllGather", Alu.bypass,
            replica_groups=[list(range(NCORES))],
            ins=[cc_in.opt()], outs=[cc_out.opt()],
        )
        bcG = singles.tile([P, 2 * NCORES], f32, name="bcG")
        nc.sync.dma_start(out=bcG, in_=bass.AP(tensor=cc_out.tensor,
                                               offset=cc_out.offset,
                                               ap=[[0, P], [1, 2 * NCORES]]))
        uu = singles.tile([P, 2], f32, name="uu")
        nc.vector.tensor_reduce(uu, bcG.rearrange("p (r i) -> p i r", i=2),
                                axis=AxX, op=Alu.add)

        # global stats -> per-channel scale/bias (column j = channels j*128+p)
        muvar = singles.tile([P, 2], f32, name="muvar")
        nc.vector.tensor_scalar_mul(muvar, uu, 1.0 / BN_COUNT)
        muv = muvar[:, 0:1]
        musq = singles.tile([P, 1], f32, name="musq")
        nc.vector.tensor_mul(musq, muv, muv)
        varv = singles.tile([P, 1], f32, name="varv")
        nc.vector.tensor_sub(varv, muvar[:, 1:2], musq)
        dv = singles.tile([P, J], f32, name="dv")
        nc.vector.tensor_scalar(dv, ww2, varv, 1e-5, op0=Alu.mult, op1=Alu.add)
        nc.scalar.activation(out=dv, in_=dv, func=Act.Sqrt)
        rst = singles.tile([P, J], f32, name="rst")
        nc.vector.reciprocal(rst, dv)
        scale2 = singles.tile([P, J], f32, name="scale2")
        nc.vector.tensor_mul(scale2, n1, rst)
        alpha = singles.tile([P, J], f32, name="alpha")
        nc.vector.tensor_mul(alpha, alw, rst)
        bias2 = singles.tile([P, J], f32, name="bias2")
        nc.vector.tensor_scalar(bias2, alpha, muv, None, op0=Alu.mult)
        nc.vector.tensor_sub(bias2, bnb, bias2)

        # out = x + scale2[c]*theta_bc + bias2[c]; quarter tiles so the
        # pipeline fills fast, stores alternate across BOTH HWDGE rings
        for j in range(J):
            for q in range(4):
                sl = slice(q * QUART, (q + 1) * QUART)
                tmp = tmps.tile([P, QUART], f32, name="tmp")
                nc.scalar.activation(out=tmp, in_=theta_bc[:, sl],
                                     func=Act.Identity,
                                     scale=scale2[:, j:j + 1],
                                     bias=bias2[:, j:j + 1])
                nc.vector.tensor_add(x_tiles[j][:, sl], x_tiles[j][:, sl],
                                     tmp)
                eng = nc.sync if (j * 4 + q) % 2 == 0 else nc.scalar
                eng.dma_start(out=out_d[j * P:(j + 1) * P, sl],
                              in_=x_tiles[j][:, sl])

    nc.compile()
    return nc


def kernel(**inputs):
    from concourse import bass_utils

    nc = _cache.get("nc")
    if nc is None:
        nc = _build_nc()
        _cache["nc"] = nc

    def f32c(a):
        return np.ascontiguousarray(np.asarray(a, dtype=np.float32))

    xs = f32c(inputs["x"]).reshape(B, C, N)
    x1s = f32c(inputs["x1"]).reshape(B, C, N)
    x2s = f32c(inputs["x2"]).reshape(B, C, N)
    shared = {
        "theta_w": f32c(inputs["theta_w"]),
        "g_w": f32c(inputs["g_w"]),
        "phi_w": f32c(inputs["phi_w"]),
        "theta_b": f32c(inputs["theta_b"]),
        "g_b": f32c(inputs["g_b"]),
        "phi_b": f32c(inputs["phi_b"]),
        "W_w": f32c(inputs["W_w"]),
        "bn_w": f32c(inputs["bn_w"]),
        "bn_b": f32c(inputs["bn_b"]),
        "a": f32c(inputs["a"]),
        "b": f32c(inputs["b"]),
    }
    in_maps = [
        {"x": xs[c], "x1": x1s[c], "x2": x2s[c], **shared}
        for c in range(NCORES)
    ]
    res = bass_utils.run_bass_kernel_spmd(
        nc, in_maps, core_ids=list(range(NCORES)),
        trace=bool(os.environ.get("BASS_TRACE")),
        tmpdir=os.environ.get("KERNEL_TMPDIR") or None,
    )
    _cache["last_results"] = res
    out = np.stack([res.results[c]["out"] for c in range(NCORES)], axis=0)
    return out.reshape(B, C, H, W)
